# revision 31
# baseline (speedup 1.0000x reference)
"""DifferentialAttention Trainium2 kernel (8-core SPMD).

Sharding: 8 cores = 4 batches x 2 head-groups (8 heads each).
Each core computes, for its (batch, head-group):
  - x^T via PE transpose
  - Q^T, K^T, V projections (f32r matmuls)
  - per head: causal scores^T (row-packed s1/s2), exp on ACT, masked via
    gpsimd affine_select, u = [V|1]^T @ p (denominator folded into the
    matmul), softmax division via K=1 broadcast matmuls, GroupNorm via
    bn_stats + cross-partition ones-matmul, ln/exp rsqrt trick
  - partial output = yn^T rows @ Wc[group rows]  (row-sharded c_proj)
Host sums the two partials per batch (the "all-reduce after").
"""

import math
import sys

for _p in ("/opt/trn_rl_repo", "/root/.axon_site/_ro/trn_rl_repo"):
    if _p not in sys.path:
        sys.path.append(_p)

from contextlib import ExitStack

import numpy as np

import concourse.mybir as mybir
import concourse.tile as tile
from concourse import bacc
from concourse.bass_utils import run_bass_kernel_spmd

F32 = mybir.dt.float32
F32R = mybir.dt.float32r
AF = mybir.ActivationFunctionType
OP = mybir.AluOpType

B, T, C = 4, 1024, 1024
NH = 16
HD = C // NH  # 64
NHL = 8  # heads per core
LAMBDA_INIT = 0.8 - 0.6 * math.exp(-0.3 * 1.0)
EPS = 1e-5
SCALE = 1.0 / math.sqrt(HD)
N_CORES = 8
NKT = T // 128  # 8 tk tiles
NKC = C // 128  # 8 contraction tiles


def _const(nc, val, shape):
    return nc.const_aps.tensor(val, shape)


def _bcast64(dram_tile):
    import concourse.bass as bass

    ap = dram_tile[:]
    return bass.AP(tensor=ap.tensor, offset=ap.offset, ap=[[0, 64], [1, 512]])


def build_program(n_iters: int = 1, stop: str = "full"):
    nc = bacc.Bacc("TRN2", target_bir_lowering=False, debug=False)
    x_d = nc.dram_tensor("xbT", [C, T], F32, kind="ExternalInput").ap()
    wq_d = nc.dram_tensor("wq", [C, 1024], F32, kind="ExternalInput").ap()
    wk_d = nc.dram_tensor("wk", [C, 1024], F32, kind="ExternalInput").ap()
    wv_d = nc.dram_tensor("wv", [C, 512], F32, kind="ExternalInput").ap()
    wc_d = nc.dram_tensor("wc", [512, C], F32, kind="ExternalInput").ap()
    neglam_d = nc.dram_tensor("neglam", [1, 64], F32, kind="ExternalInput").ap()
    out_d = nc.dram_tensor("outp", [T, C], F32, kind="ExternalOutput").ap()

    with tile.TileContext(nc) as tc, ExitStack() as ctx:
        for _ in range(n_iters):
            _emit_iteration(nc, tc, x_d, wq_d, wk_d, wv_d, wc_d, neglam_d, out_d, stop)

    nc.compile()
    return nc


def _emit_iteration(nc, tc, x_d, wq_d, wk_d, wv_d, wc_d, neglam_d, out_d, stop="full"):
    with ExitStack() as ctx:
        # ---------------- long-lived pools ----------------
        lp = ctx.enter_context(tc.tile_pool(name="long", bufs=1))
        qk = ctx.enter_context(tc.tile_pool(name="qk", bufs=1))

        # consts
        neglam_f = lp.tile([128, 64], F32, tag="neglam_f")
        nc.sync.dma_start(neglam_f[64:65, :], neglam_d)
        ones64 = lp.tile([64, 64], F32, tag="ones64")
        nc.vector.tensor_copy(ones64[:], _const(nc, 1.0, (64, 64)))
        epsc = lp.tile([64, 1], F32, tag="epsc")
        nc.vector.memset(epsc[:], EPS)
        lnb = lp.tile([64, 1], F32, tag="lnb")
        nc.vector.memset(lnb[:], float(math.log(1.0 - LAMBDA_INIT)))

        # Vaug tiles: (128, 8 heads, 65) f32r, col 64 = ones
        vaug = [lp.tile([128, NHL, HD + 1], F32R, tag=f"vaug{t}", name=f"vaug{t}") for t in range(NKT)]
        # Q^T / K^T tiles f32r
        QT = [qk.tile([128, T], F32R, tag=f"qt{m}", name=f"qt{m}") for m in range(NKC)]
        KT = [qk.tile([128, T], F32R, tag=f"kt{m}", name=f"kt{m}") for m in range(NKC)]

        # ---------------- phase A+B: transpose + projections ----------------
        with ExitStack() as ab:
            sba = ab.enter_context(tc.tile_pool(name="sba", bufs=4))
            wst = ab.enter_context(tc.tile_pool(name="wst", bufs=2))
            xtp = ab.enter_context(tc.tile_pool(name="xtp", bufs=1))
            psb = ab.enter_context(tc.tile_pool(name="psb", bufs=3, space="PSUM"))

            xT = [xtp.tile([128, T], F32R, tag=f"xt{k}", name=f"xt{k}") for k in range(NKC)]
            for cc in range(NKC):
                xf = sba.tile([128, T], F32, tag="xf")
                nc.sync.dma_start(xf[:], x_d[cc * 128 : (cc + 1) * 128, :])
                if cc % 2 == 0:
                    nc.scalar.copy(xT[cc][:], xf[:])
                else:
                    nc.vector.tensor_copy(xT[cc][:], xf[:])

            # --- weights cast helper
            def load_w(dram, k, width, tag):
                wf = wst.tile([128, width], F32, tag=f"wf_{width}", bufs=4)
                nc.sync.dma_start(wf[:], dram[k * 128 : (k + 1) * 128, :])
                wr = wst.tile([128, width], F32R, tag=f"wr_{tag}_{k}", bufs=1)
                if k % 2 == 0:
                    nc.scalar.copy(wr[:], wf[:])
                else:
                    nc.vector.tensor_copy(wr[:], wf[:])
                return wr

            # --- Q^T projection (then K^T reusing weight slots)
            for name, dram, dest in (("qk", wq_d, QT), ("qk", wk_d, KT)):
                w_r = [load_w(dram, k, 1024, name) for k in range(NKC)]
                for m in range(NKC):
                    pq = psb.tile([128, T], F32, tag="proj", bufs=3)
                    for c0 in range(0, T, 512):
                        for k in range(NKC):
                            nc.tensor.matmul(
                                pq[:, c0 : c0 + 512],
                                w_r[k][:, m * 128 : (m + 1) * 128],
                                xT[k][:, c0 : c0 + 512],
                                start=(k == 0),
                                stop=(k == NKC - 1),
                            )
                    if m % 2 == 0:
                        nc.vector.tensor_copy(dest[m][:], pq[:])
                    else:
                        nc.scalar.copy(dest[m][:], pq[:])

            # --- V projection into Vaug
            wv_r = [load_w(wv_d, k, 512, "v") for k in range(NKC)]
            for tt in range(NKT):
                pv = psb.tile([128, 512], F32, tag="projv", bufs=2)
                for k in range(NKC):
                    nc.tensor.matmul(
                        pv[:],
                        xT[k][:, tt * 128 : (tt + 1) * 128],
                        wv_r[k][:],
                        start=(k == 0),
                        stop=(k == NKC - 1),
                    )
                nc.vector.tensor_copy(
                    vaug[tt][:, :, 0:HD],
                    pv[:].rearrange("p (h d) -> p h d", h=NHL),
                )
                nc.vector.tensor_copy(vaug[tt][:, :, HD : HD + 1], _const(nc, 1.0, (128, NHL, 1)))


        if stop == "ab":
            for m in range(NKC):
                nc.sync.dma_start(out_d[m * 128 : (m + 1) * 128, :], QT[m][:].bitcast(F32))
            return

        # ---------------- phase C: attention per head ----------------
        yout = ctx.enter_context(tc.tile_pool(name="yn", bufs=1))
        with ExitStack() as cc_:
            pp = cc_.enter_context(tc.tile_pool(name="pp", bufs=1))
            yt = cc_.enter_context(tc.tile_pool(name="yt", bufs=1))
            sm = cc_.enter_context(tc.tile_pool(name="sm", bufs=2))
            pss = cc_.enter_context(tc.tile_pool(name="pss", bufs=1, space="PSUM"))
            psu = cc_.enter_context(tc.tile_pool(name="psu", bufs=2, space="PSUM"))
            drp = cc_.enter_context(tc.tile_pool(name="drp", bufs=3, space="DRAM"))

            yTn = [yout.tile([128, T], F32R, tag=f"ytn{k}", name=f"ytn{k}") for k in range(4)]


            meanAll = sm.tile([64, NHL], F32, tag="meanAll", bufs=1)
            varAll = sm.tile([64, NHL], F32, tag="varAll", bufs=1)
            yT_heads = {}
            nheads = NHL if not stop.startswith("c") else int(stop[1:])

            # persistent p tiles, reused by every head (zero-filled once)
            p1_t = [
                pp.tile([128, T - (i // 4) * 512], F32R, tag=f"p1_{i}", name=f"p1_{i}")
                for i in range(NKT)
            ]
            p2_t = [
                pp.tile([128, T - (i // 4) * 512], F32R, tag=f"p2_{i}", name=f"p2_{i}")
                for i in range(NKT)
            ]
            for i in range(NKT):
                d0 = i * 128 - (i // 4) * 512
                if d0 > 0:
                    z = _const(nc, 0.0, (128, d0))
                    nc.gpsimd.tensor_copy(p1_t[i][:, 0:d0], z)
                    nc.gpsimd.tensor_copy(p2_t[i][:, 0:d0], z)

            def emit_score_mms(j, i):
                c0 = (i // 4) * 512
                w = T - c0
                s1 = pss.tile([128, w], F32, tag="s1", name=f"s1_{j}_{i}")
                s2 = pss.tile([128, w], F32, tag="s2", name=f"s2_{j}_{i}")
                for cb in range(0, w, 512):
                    cw = min(512, w - cb)
                    nc.tensor.matmul(
                        s1[:, cb : cb + cw],
                        KT[j][0:64, i * 128 : (i + 1) * 128],
                        QT[j][0:64, c0 + cb : c0 + cb + cw],
                        start=True,
                        stop=True,
                        tile_position=(0, 0),
                    )
                    nc.tensor.matmul(
                        s2[:, cb : cb + cw],
                        KT[j][64:128, i * 128 : (i + 1) * 128],
                        QT[j][64:128, c0 + cb : c0 + cb + cw],
                        start=True,
                        stop=True,
                        tile_position=(64, 0),
                    )
                return s1, s2

            def emit_scores(j, s_pre=None):
                for i in range(NKT):
                    c0 = (i // 4) * 512  # chunk base for this k-tile
                    w = T - c0  # p tile width (1024 or 512)
                    d0 = i * 128 - c0  # diag offset within tile
                    if i == 0 and s_pre is not None:
                        s1, s2 = s_pre
                    else:
                        s1, s2 = emit_score_mms(j, i)
                    p1t = p1_t[i]
                    p2t = p2_t[i]
                    nc.scalar.activation(p1t[:, d0:], s1[:, d0:], AF.Exp, scale=SCALE)
                    nc.scalar.activation(p2t[:, d0:], s2[:, d0:], AF.Exp, scale=SCALE)
                    for pt in (p1t, p2t):
                        nc.gpsimd.affine_select(
                            out=pt[:, d0 : d0 + 128],
                            in_=pt[:, d0 : d0 + 128],
                            compare_op=OP.is_ge,
                            fill=0.0,
                            base=0,
                            pattern=[[1, 128]],
                            channel_multiplier=-1,
                        )
                yT_heads[j] = yt.tile([64, T], F32, tag=f"yT{j}", name=f"yTh{j}")

            def emit_u(j, c):
                p1, p2 = p1_t, p2_t
                yT_h = yT_heads[j]
                ilast = min(NKT, (c + 1) * 4) - 1
                u1 = psu.tile([HD + 1, 512], F32, tag="u1", name=f"u1_{j}_{c}")
                u2 = psu.tile([HD + 1, 512], F32, tag="u2", name=f"u2_{j}_{c}")
                for i in range(ilast + 1):
                    lo = c * 512 - (i // 4) * 512  # chunk start in p-tile coords
                    nc.tensor.matmul(
                        u1[:],
                        vaug[i][:, j, :],
                        p1[i][:, lo : lo + 512],
                        start=(i == 0),
                        stop=(i == ilast),
                    )
                for i in range(ilast + 1):
                    lo = c * 512 - (i // 4) * 512
                    nc.tensor.matmul(
                        u2[:],
                        vaug[i][:, j, :],
                        p2[i][:, lo : lo + 512],
                        start=(i == 0),
                        stop=(i == ilast),
                    )
                rr1 = sm.tile([128, 512], F32, tag="rr1", name=f"rr1_{j}_{c}")
                rr2 = sm.tile([128, 512], F32, tag="rr2", name=f"rr2_{j}_{c}")
                nc.vector.reciprocal(rr1[64:65, :], u1[64:65, :])
                nc.vector.reciprocal(rr2[64:65, :], u2[64:65, :])
                nc.vector.tensor_scalar_mul(
                    rr2[64:65, :], rr2[64:65, :], neglam_f[64:65, 0:1]
                )
                db1 = drp.tile([1, 512], F32, tag="db1", name=f"db1_{j}_{c}")
                db2 = drp.tile([1, 512], F32, tag="db2", name=f"db2_{j}_{c}")
                nc.sync.dma_start(db1[:], rr1[64:65, :])
                nc.sync.dma_start(db2[:], rr2[64:65, :])
                R1s = sm.tile([64, 512], F32, tag="R1s", name=f"R1s_{j}_{c}")
                R2s = sm.tile([64, 512], F32, tag="R2s", name=f"R2s_{j}_{c}")
                nc.sync.dma_start(R1s[:], _bcast64(db1))
                nc.sync.dma_start(R2s[:], _bcast64(db2))
                t1 = sm.tile([64, 512], F32, tag="t1", name=f"t1_{j}_{c}")
                t2 = sm.tile([64, 512], F32, tag="t2", name=f"t2_{j}_{c}")
                nc.vector.tensor_tensor(t1[:], u1[0:HD, :], R1s[:], OP.mult)
                nc.vector.tensor_tensor(t2[:], u2[0:HD, :], R2s[:], OP.mult)
                nc.vector.tensor_tensor(yT_h[:, c * 512 : (c + 1) * 512], t1[:], t2[:], OP.add)

            def emit_stats(j):
                yT_h = yT_heads[j]
                bstats = sm.tile([64, 2, 6], F32, tag="bst", name=f"bst_{j}")
                for si in range(2):
                    nc.vector.bn_stats(out=bstats[:, si, :], in_=yT_h[:, si * 512 : (si + 1) * 512])
                mv = sm.tile([64, 2], F32, tag="mv", name=f"mv_{j}")
                nc.vector.bn_aggr(out=mv[:], in_=bstats[:])
                st = sm.tile([64, 2], F32, tag="st", name=f"st_{j}")
                m2p = sm.tile([64, 1], F32, tag="m2p", name=f"m2p_{j}")
                nc.vector.tensor_tensor(m2p[:], mv[:, 0:1], mv[:, 0:1], OP.mult)
                nc.vector.tensor_tensor(st[:, 1:2], mv[:, 1:2], m2p[:], OP.add)
                nc.vector.tensor_copy(st[:, 0:1], mv[:, 0:1])
                pstat = psu.tile([64, 2], F32, tag="u1", name=f"pstat_{j}")
                nc.tensor.matmul(pstat[:], ones64[:], st[:], start=True, stop=True)
                stats = sm.tile([64, 2], F32, tag="stats", name=f"stats_{j}")
                nc.vector.tensor_scalar_mul(stats[:], pstat[:], 1.0 / 64.0)
                nc.vector.tensor_copy(meanAll[:, j : j + 1], stats[:, 0:1])
                m2 = sm.tile([64, 1], F32, tag="m2", name=f"m2_{j}")
                nc.vector.tensor_tensor(m2[:], stats[:, 0:1], stats[:, 0:1], OP.mult)
                nc.vector.tensor_tensor(varAll[:, j : j + 1], stats[:, 1:2], m2[:], OP.subtract)

            # software-pipelined head loop. u(j-1, c1) is emitted BEFORE
            # scores/exps(j) so the WAR on the shared p tiles orders exp(j)
            # after u(j-1)'s reads (per-tile granularity keeps the overlap).
            for j in range(nheads):
                s_pre = None
                if j > 0:
                    s_pre = emit_score_mms(j, 0)
                    emit_u(j - 1, 1)
                emit_scores(j, s_pre)
                emit_u(j, 0)
                if j > 0:
                    emit_stats(j - 1)
            emit_u(nheads - 1, 1)
            emit_stats(nheads - 1)

            # ---- batched groupnorm: ln/exp once, then normalize all heads
            lnvAll = sm.tile([64, NHL], F32, tag="lnvAll", bufs=1)
            nc.scalar.activation(lnvAll[:, 0:nheads], varAll[:, 0:nheads], AF.Ln, bias=epsc[:])
            rstdAll = sm.tile([64, NHL], F32, tag="rstdAll", bufs=1)
            nc.scalar.activation(
                rstdAll[:, 0:nheads], lnvAll[:, 0:nheads], AF.Exp, scale=-0.5, bias=lnb[:]
            )
            for j in range(nheads):
                if j % 2 == 0:
                    nc.vector.tensor_scalar(
                        out=yTn[j // 2][0:64, :],
                        in0=yT_heads[j][:],
                        scalar1=meanAll[:, j : j + 1],
                        scalar2=rstdAll[:, j : j + 1],
                        op0=OP.subtract,
                        op1=OP.mult,
                    )
                else:
                    ymv = sm.tile([64, T], F32R, tag="ymv", bufs=1)
                    nc.vector.tensor_scalar(
                        out=ymv[:],
                        in0=yT_heads[j][:],
                        scalar1=meanAll[:, j : j + 1],
                        scalar2=rstdAll[:, j : j + 1],
                        op0=OP.subtract,
                        op1=OP.mult,
                    )
                    nc.sync.dma_start(yTn[j // 2][64:128, :], ymv[:])

            if stop.startswith("c"):
                for k in range(nheads // 2):
                    nc.sync.dma_start(
                        out_d[k * 128 : (k + 1) * 128, :], yTn[k][:].bitcast(F32)
                    )
                return

        # ---------------- phase E: output projection ----------------
        with ExitStack() as ee:
            oe = ee.enter_context(tc.tile_pool(name="oe", bufs=2))
            pso = ee.enter_context(tc.tile_pool(name="pso", bufs=3, space="PSUM"))
            wc_r = []
            for k in range(4):
                wcf = oe.tile([128, C], F32, tag="wcf", bufs=1)
                nc.sync.dma_start(wcf[:], wc_d[k * 128 : (k + 1) * 128, :])
                wr = oe.tile([128, C], F32R, tag=f"wc{k}", bufs=1, name=f"wcr{k}")
                nc.scalar.copy(wr[:], wcf[:])
                wc_r.append(wr)
            for m in range(NKC):
                po = pso.tile([128, C], F32, tag="o")
                for c0 in range(0, C, 512):
                    for kk in range(4):
                        nc.tensor.matmul(
                            po[:, c0 : c0 + 512],
                            yTn[kk][:, m * 128 : (m + 1) * 128],
                            wc_r[kk][:, c0 : c0 + 512],
                            start=(kk == 0),
                            stop=(kk == 3),
                        )
                osb = oe.tile([128, C], F32, tag="osb")
                if m % 2 == 0:
                    nc.vector.tensor_copy(osb[:], po[:])
                else:
                    nc.scalar.copy(osb[:], po[:])
                nc.sync.dma_start(out_d[m * 128 : (m + 1) * 128, :], osb[:])


_PROGRAM_CACHE = {}


def get_program(n_iters: int = 1):
    if n_iters not in _PROGRAM_CACHE:
        _PROGRAM_CACHE[n_iters] = build_program(n_iters)
    return _PROGRAM_CACHE[n_iters]


def make_in_maps(x, Wq, Wk, Wv, Wc, lambda_q1, lambda_k1, lambda_q2, lambda_k2):
    lam = (
        math.exp(float(np.sum(lambda_q1.astype(np.float64) * lambda_k1.astype(np.float64))))
        - math.exp(float(np.sum(lambda_q2.astype(np.float64) * lambda_k2.astype(np.float64))))
        + LAMBDA_INIT
    )
    neglam = np.full((1, 64), -lam, dtype=np.float32)
    in_maps = []
    for core in range(N_CORES):
        b, g = core // 2, core % 2
        in_maps.append(
            {
                "xbT": np.ascontiguousarray(x[b].T),
                "wq": np.ascontiguousarray(Wq[:, g * 1024 : (g + 1) * 1024]),
                "wk": np.ascontiguousarray(Wk[:, g * 1024 : (g + 1) * 1024]),
                "wv": np.ascontiguousarray(Wv[:, g * 512 : (g + 1) * 512]),
                "wc": np.ascontiguousarray(Wc[g * 512 : (g + 1) * 512, :]),
                "neglam": neglam,
            }
        )
    return in_maps


def kernel(x, Wq, Wk, Wv, Wc, lambda_q1, lambda_k1, lambda_q2, lambda_k2):
    x = np.asarray(x, dtype=np.float32)
    in_maps = make_in_maps(
        x,
        np.asarray(Wq, np.float32),
        np.asarray(Wk, np.float32),
        np.asarray(Wv, np.float32),
        np.asarray(Wc, np.float32),
        np.asarray(lambda_q1, np.float32),
        np.asarray(lambda_k1, np.float32),
        np.asarray(lambda_q2, np.float32),
        np.asarray(lambda_k2, np.float32),
    )
    nc = get_program(1)
    res = run_bass_kernel_spmd(nc, in_maps, list(range(N_CORES)))
    out = np.empty((B, T, C), dtype=np.float32)
    for b in range(B):
        out[b] = res.results[2 * b]["outp"] + res.results[2 * b + 1]["outp"]
    return out
